# revision 1
# baseline (speedup 1.0000x reference)
"""Single-head attention with QKV projections for TRN2, batch-sharded across
8 NeuronCores (one batch element per core).

Reference computation per batch element (S=2048, D=1024, fp32):
    Q = xq @ Wq + bq ; K = xk @ Wk + bk ; V = xv @ Wv + bv
    L = Q @ K^T                      # [S, S]
    out = (softmax(L, -1) * 1/sqrt(D)) @ V

Per-core kernel plan (all matmuls fp32r = full-rate fp32 on the PE):
  Phase A-Q: xq tiles -> PE-transpose -> xq^T ; Q^T = Wq^T @ xq^T -> DRAM scratch
  Phase A-K: K^T = Wk^T @ xk^T        -> resident SBUF [1024, 2048]
  Phase A-V: V = xv @ Wv              -> resident SBUF [2048, 1024]
  Phase B (per 128-row q tile): L row-strip = (Q^T)^T tiles @ K^T (N=512 moving),
    exp on ACT straight out of PSUM (no max subtraction: |L| < ~80 so exp fits
    fp32), row-sum on DVE, P^T via PE transpose, out = P^T.T @ V accumulated in
    PSUM, normalized by (1/32)/rowsum via per-partition tensor_scalar.
"""
import numpy as np
from contextlib import ExitStack

import concourse.bass as bass
import concourse.bacc as bacc
import concourse.tile as tile
import concourse.mybir as mybir
from concourse.bass_utils import run_bass_kernel_spmd

F32 = mybir.dt.float32
F32R = mybir.dt.float32r
AF = mybir.ActivationFunctionType

B, S, D = 8, 2048, 1024
NKT = D // 128          # 8 contraction tiles
NST = S // 128          # 16 s tiles
SCALE = 1.0 / 32.0      # 1/sqrt(D)

_CACHED = {}


def build():
    nc = bacc.Bacc("TRN2", target_bir_lowering=False, debug=False, num_devices=8)

    xq = nc.dram_tensor("xq", [S, D], F32, kind="ExternalInput")
    xk = nc.dram_tensor("xk", [S, D], F32, kind="ExternalInput")
    xv = nc.dram_tensor("xv", [S, D], F32, kind="ExternalInput")
    wq = nc.dram_tensor("wq", [D, D], F32R, kind="ExternalInput")
    wk = nc.dram_tensor("wk", [D, D], F32R, kind="ExternalInput")
    wv = nc.dram_tensor("wv", [D, D], F32R, kind="ExternalInput")
    bqd = nc.dram_tensor("bqd", [128, NKT], F32, kind="ExternalInput")  # bq.reshape(8,128).T
    bkd = nc.dram_tensor("bkd", [128, NKT], F32, kind="ExternalInput")
    bvd = nc.dram_tensor("bvd", [1, D], F32R, kind="ExternalInput")
    identd = nc.dram_tensor("identd", [128, 128], F32, kind="ExternalInput")
    ones1d = nc.dram_tensor("ones1d", [1, 128], F32R, kind="ExternalInput")

    out = nc.dram_tensor("out", [S, D], F32, kind="ExternalOutput")
    qt_dram = nc.dram_tensor("qt_scratch", [D, S], F32R)  # internal scratch

    with tile.TileContext(nc) as tc, ExitStack() as ctx:
        # ---------------- persistent pools ----------------
        cpool = ctx.enter_context(tc.tile_pool(name="const", bufs=1))
        ktp = ctx.enter_context(tc.tile_pool(name="ktr", bufs=1))
        vsp = ctx.enter_context(tc.tile_pool(name="vres", bufs=1))
        pp = ctx.enter_context(tc.tile_pool(name="pp", bufs=2, space="PSUM"))
        tp = ctx.enter_context(tc.tile_pool(name="tp", bufs=2, space="PSUM"))
        op = ctx.enter_context(tc.tile_pool(name="op", bufs=3, space="PSUM"))

        ident = cpool.tile([128, 128], F32, tag="ident")
        bqs = cpool.tile([128, NKT], F32, tag="bqs")
        bks = cpool.tile([128, NKT], F32, tag="bks")
        bvs = cpool.tile([1, D], F32R, tag="bvs")
        ones1 = cpool.tile([1, 128], F32R, tag="ones1")
        bvb = cpool.tile([128, D], F32, tag="bvb")
        nc.sync.dma_start(ident[:], identd.ap())
        nc.sync.dma_start(bqs[:], bqd.ap())
        nc.sync.dma_start(bks[:], bkd.ap())
        nc.sync.dma_start(bvs[:], bvd.ap())
        nc.sync.dma_start(ones1[:], ones1d.ap())

        # broadcast bv across partitions via K=1 matmul: bvb = ones1.T @ bvs
        for h in range(2):
            bps = op.tile([128, 512], F32, tag="av")
            nc.tensor.matmul(bps[:], ones1[:], bvs[:, h * 512:(h + 1) * 512],
                             start=True, stop=True)
            nc.scalar.copy(bvb[:, h * 512:(h + 1) * 512], bps[:])

        kt = ktp.tile([128, NKT * S], F32R, tag="kt")       # K^T resident
        vs = vsp.tile([128, NST * D], F32R, tag="vs")       # V resident

        # ---------------- phase A: projections ----------------
        def load_w(wpool, w_dram):
            w_s = wpool.tile([128, NKT * D], F32R, tag="w")
            for k in range(NKT):
                nc.sync.dma_start(w_s[:, k * D:(k + 1) * D],
                                  w_dram.ap()[k * 128:(k + 1) * 128, :])
            return w_s

        def transpose_strip(xpool, xtpool, x_dram, j, n_stiles):
            """Load x rows [j*128*n .. ) and produce x^T strip [D, 128*n] (f32r)."""
            xt = xtpool.tile([128, NKT * 128 * n_stiles], F32R, tag="xt")
            for st in range(n_stiles):
                xl = xpool.tile([128, D], F32, tag="xl")
                nc.sync.dma_start(
                    xl[:], x_dram.ap()[(j * n_stiles + st) * 128:(j * n_stiles + st + 1) * 128, :])
                for k4 in range(NKT // 4):
                    tpt = tp.tile([128, 512], F32, tag="tp")
                    for kk in range(4):
                        k = k4 * 4 + kk
                        nc.tensor.transpose(tpt[:, kk * 128:(kk + 1) * 128],
                                            xl[:, k * 128:(k + 1) * 128], ident[:])
                    # scatter 4 transposed tiles into xt at (k, st) slots
                    dst = xt[:].rearrange("p (k s) -> p k s", s=128 * n_stiles)
                    nc.scalar.copy(
                        dst[:, k4 * 4:k4 * 4 + 4, st * 128:(st + 1) * 128], tpt[:])
            return xt

        with ExitStack() as actx:
            wpool = actx.enter_context(tc.tile_pool(name="wpool", bufs=1))
            xpool = actx.enter_context(tc.tile_pool(name="xpool", bufs=2))
            xtpool = actx.enter_context(tc.tile_pool(name="xtpool", bufs=1))
            qstg = actx.enter_context(tc.tile_pool(name="qstg", bufs=3))

            # ---- A-Q: Q^T -> DRAM scratch ----
            with nc.named_scope("phase_aq"):
                w_s = load_w(wpool, wq)
                for j in range(4):
                    xt = transpose_strip(xpool, xtpool, xq, j, 4)
                    for m in range(NKT):
                        ppt = pp.tile([128, 512], F32, tag="pp")
                        for k in range(NKT):
                            nc.tensor.matmul(
                                ppt[:],
                                w_s[:, k * D + m * 128:k * D + (m + 1) * 128],
                                xt[:, k * 512:(k + 1) * 512],
                                start=(k == 0), stop=(k == NKT - 1))
                        qs_t = qstg.tile([128, 512], F32R, tag="qs")
                        nc.scalar.activation(qs_t[:], ppt[:], AF.Identity,
                                             bias=bqs[:, m:m + 1])
                        nc.sync.dma_start(
                            qt_dram.ap()[m * 128:(m + 1) * 128, j * 512:(j + 1) * 512],
                            qs_t[:])

            # ---- A-K: K^T resident ----
            with nc.named_scope("phase_ak"):
                w_s = load_w(wpool, wk)
                for j in range(4):
                    xt = transpose_strip(xpool, xtpool, xk, j, 4)
                    for m in range(NKT):
                        ppt = pp.tile([128, 512], F32, tag="pp")
                        for k in range(NKT):
                            nc.tensor.matmul(
                                ppt[:],
                                w_s[:, k * D + m * 128:k * D + (m + 1) * 128],
                                xt[:, k * 512:(k + 1) * 512],
                                start=(k == 0), stop=(k == NKT - 1))
                        nc.scalar.activation(
                            kt[:, m * S + j * 512:m * S + (j + 1) * 512],
                            ppt[:], AF.Identity, bias=bks[:, m:m + 1])

            # ---- A-V: V resident ----
            with nc.named_scope("phase_av"):
                w_s = load_w(wpool, wv)
                for j in range(4):
                    xt = transpose_strip(xpool, xtpool, xv, j, 4)
                    for m in range(4):          # s tiles within strip
                        sg = j * 4 + m
                        for h in range(2):      # dout halves
                            ppt = pp.tile([128, 512], F32, tag="pp")
                            for k in range(NKT):
                                nc.tensor.matmul(
                                    ppt[:],
                                    xt[:, k * 512 + m * 128:k * 512 + (m + 1) * 128],
                                    w_s[:, k * D + h * 512:k * D + (h + 1) * 512],
                                    start=(k == 0), stop=(k == NKT - 1))
                            nc.scalar.copy(
                                vs[:, sg * D + h * 512:sg * D + (h + 1) * 512], ppt[:])

        # ---------------- phase B: attention ----------------
        with ExitStack() as bctx, nc.named_scope("phase_b"):
            qsp = bctx.enter_context(tc.tile_pool(name="qsp", bufs=2))
            psp = bctx.enter_context(tc.tile_pool(name="psp", bufs=2))
            ptp = bctx.enter_context(tc.tile_pool(name="ptp", bufs=2))
            osp = bctx.enter_context(tc.tile_pool(name="osp", bufs=2))
            rsp = bctx.enter_context(tc.tile_pool(name="rsp", bufs=2))

            for j in range(8):                  # q strips of 256
                qs = qsp.tile([128, NKT * 256], F32R, tag="qs")
                src = qt_dram.ap()[:, j * 256:(j + 1) * 256]
                nc.sync.dma_start(
                    qs[:].rearrange("p (k s) -> p k s", s=256),
                    src.rearrange("(k p) s -> p k s", p=128))
                for m in range(2):              # q tiles of 128
                    sq = j * 2 + m
                    pstr = psp.tile([128, S], F32, tag="pstr")
                    for n in range(4):          # sk blocks of 512
                        lpt = pp.tile([128, 512], F32, tag="pp")
                        for k in range(NKT):
                            nc.tensor.matmul(
                                lpt[:],
                                qs[:, k * 256 + m * 128:k * 256 + (m + 1) * 128],
                                kt[:, k * S + n * 512:k * S + (n + 1) * 512],
                                start=(k == 0), stop=(k == NKT - 1))
                        nc.scalar.activation(pstr[:, n * 512:(n + 1) * 512],
                                             lpt[:], AF.Exp)
                    # row sums -> (1/32) / rowsum
                    rst = rsp.tile([128, 1], F32, tag="rst")
                    rct = rsp.tile([128, 1], F32, tag="rct")
                    nc.vector.reduce_sum(rst[:], pstr[:], axis=mybir.AxisListType.X)
                    nc.vector.reciprocal(rct[:], rst[:])
                    nc.vector.tensor_scalar_mul(rct[:], rct[:], SCALE)
                    # P^T via PE transposes
                    pt = ptp.tile([128, NST * 128], F32R, tag="pt")
                    for n4 in range(4):
                        tpt = tp.tile([128, 512], F32, tag="tp")
                        for tt in range(4):
                            t = n4 * 4 + tt
                            nc.tensor.transpose(tpt[:, tt * 128:(tt + 1) * 128],
                                                pstr[:, t * 128:(t + 1) * 128],
                                                ident[:])
                        nc.scalar.copy(pt[:, n4 * 512:(n4 + 1) * 512], tpt[:])
                    # out = P^T.T @ V, normalized
                    os_t = osp.tile([128, D], F32, tag="os")
                    for h in range(2):
                        opt = op.tile([128, 512], F32, tag="av")
                        for t in range(NST):
                            nc.tensor.matmul(
                                opt[:],
                                pt[:, t * 128:(t + 1) * 128],
                                vs[:, t * D + h * 512:t * D + (h + 1) * 512],
                                start=(t == 0), stop=(t == NST - 1))
                        nc.vector.tensor_scalar_mul(
                            os_t[:, h * 512:(h + 1) * 512], opt[:], rct[:])
                    nc.vector.tensor_add(os_t[:], os_t[:], bvb[:])
                    nc.sync.dma_start(out.ap()[sq * 128:(sq + 1) * 128, :], os_t[:])

    nc.compile()
    return nc


def _get_nc():
    if "nc" not in _CACHED:
        _CACHED["nc"] = build()
    return _CACHED["nc"]


def kernel(q, k, v, Wq, bq, Wk, bk, Wv, bv, _trace=False, _trace_kwargs=None):
    q = np.ascontiguousarray(q, np.float32)
    k = np.ascontiguousarray(k, np.float32)
    v = np.ascontiguousarray(v, np.float32)
    consts = {
        "wq": np.ascontiguousarray(Wq, np.float32),
        "wk": np.ascontiguousarray(Wk, np.float32),
        "wv": np.ascontiguousarray(Wv, np.float32),
        "bqd": np.ascontiguousarray(np.asarray(bq, np.float32).reshape(NKT, 128).T),
        "bkd": np.ascontiguousarray(np.asarray(bk, np.float32).reshape(NKT, 128).T),
        "bvd": np.asarray(bv, np.float32).reshape(1, D).copy(),
        "identd": np.eye(128, dtype=np.float32),
        "ones1d": np.ones((1, 128), np.float32),
    }
    in_maps = [dict(consts, xq=q[c], xk=k[c], xv=v[c]) for c in range(B)]

    nc = _get_nc()
    res = run_bass_kernel_spmd(nc, in_maps, core_ids=list(range(B)),
                               trace=_trace, **(_trace_kwargs or {}))
    out = np.stack([res.results[c]["out"] for c in range(B)])
    if _trace:
        kernel.last_results = res
    return out
